# revision 9
# baseline (speedup 1.0000x reference)
import sys

sys.path.insert(0, "/opt/trn_rl_repo")

from contextlib import ExitStack

import ml_dtypes
import numpy as np

import concourse.bass as bass
import concourse.bacc as bacc
import concourse.tile as tile
from concourse import bass_utils, mybir

BF16 = mybir.dt.bfloat16
F32 = mybir.dt.float32
AF = mybir.ActivationFunctionType
ALU = mybir.AluOpType
NPBF16 = ml_dtypes.bfloat16

B, C, H, W = 16, 256, 64, 64
Cq = C // 4        # 64
Gh = C // 2        # 128
P = H * W          # 4096 spatial positions per sample
MARG = 66          # zero margin (elements) on each side of the flat spatial dim
NCORES = 8
S = 2              # samples per core
NT = 1024          # psum tile free size
EPS = 1e-5

# ---- weight blob column offsets (bf16 blob, [128, WB]) ----
W1_O = 0                       # [9 taps][2 kb][64 o]            -> 1152
W1C_O = W1_O + 9 * 2 * 64      # [2 edge][3 dy][2 kb][64 o]      -> 768
W2_O = W1C_O + 2 * 3 * 2 * 64  # [9 taps][64 o] (dup both halves)-> 576
W2C_O = W2_O + 9 * 64          # [2 edge][3 dy][64 o] (dup)      -> 384
OW_O = W2C_O + 2 * 3 * 64      # [256] out_w.T (dup)             -> 256
G1A_O = OW_O + 256             # [128]
G1B_O = G1A_O + 128            # [128]
G1C_O = G1B_O + 128            # [128] (dup halves)
G2_O = G1C_O + 128             # [256]
ONES_O = G2_O + 256            # [64] all ones
OB4_O = ONES_O + 64            # [4]  out_b per (Mb*2+s) col
WB = OB4_O + 4

# f32 vector blob [128, VB]: bias1, bias2, hb, g2b(2 cols)
VB = 5


def _prep_weights(i):
    f = np.float32
    inv1 = (i["bn1_g"] / np.sqrt(i["bn1_v"] + EPS)).astype(f)          # [64]
    W1f = (i["ec1_w"] * inv1[:, None, None, None]).astype(f)           # [64,256,3,3]
    bias1 = (inv1 * (i["ec1_b"] - i["bn1_m"]) + i["bn1_b"]).astype(f)  # [64]

    inv2 = (i["bn2_g"] / np.sqrt(i["bn2_v"] + EPS)).astype(f)
    W2f = (i["ec2_w"] * inv2[:, None, None, None]).astype(f)           # [64,64,3,3]
    bias2 = (inv2 * (i["ec2_b"] - i["bn2_m"]) + i["bn2_b"]).astype(f)

    pre = (i["gbn_g"] / np.sqrt(i["gbn_v"] + EPS)).astype(f)           # [128]
    G1f = (i["g1_w"] * pre[:, None] / P).astype(f)                     # [128,320]
    hb = (pre * (i["g1_b"] - i["gbn_m"]) + i["gbn_b"]).astype(f)       # [128]

    wb = np.zeros((128, WB), dtype=f)
    # w1: [k, t9, kb, o] = W1f[o, kb*128+k, dy, dx]
    a = np.transpose(W1f.reshape(64, 2, 128, 3, 3), (2, 3, 4, 1, 0))  # [k,dy,dx,kb,o]
    wb[:, W1_O:W1C_O] = a.reshape(128, 9 * 2 * 64)
    # w1c: [k, e, dy, kb, o] = -W1f[o, kb*128+k, dy, dx_e], dx_e = (0, 2)
    an = -np.transpose(W1f.reshape(64, 2, 128, 3, 3), (2, 3, 1, 0, 4))  # [k,dy,kb,o,dx]
    w1c = np.stack([an[..., 0], an[..., 2]], axis=1)  # [k, e, dy, kb, o]
    wb[:, W1C_O:W2_O] = w1c.reshape(128, 768)
    # w2: [64s+k, t9, o] = W2f[o, k, dy, dx]
    b2 = np.transpose(W2f, (1, 2, 3, 0)).reshape(64, 9, 64)  # [k, t9, o]
    wb[:, W2_O:W2C_O] = np.concatenate([b2, b2], axis=0).reshape(128, 576)
    # w2c: [64s+k, e, dy, o] = -W2f[o, k, dy, dx_e]
    b2n = -np.transpose(W2f, (1, 2, 3, 0))  # [k, dy, dx, o]
    w2c = np.stack([b2n[:, :, 0, :], b2n[:, :, 2, :]], axis=1)  # [k, e, dy, o]
    wb[:, W2C_O:OW_O] = np.concatenate([w2c, w2c], axis=0).reshape(128, 384)
    # out_w.T dup
    owT = i["out_w"].astype(f).T  # [64, 256]
    wb[:, OW_O:G1A_O] = np.concatenate([owT, owT], axis=0)
    # gating
    wb[:, G1A_O:G1B_O] = G1f[:, 0:128].T
    wb[:, G1B_O:G1C_O] = G1f[:, 128:256].T
    g1c = G1f[:, 256:320].T  # [64, 128]
    wb[:, G1C_O:G2_O] = np.concatenate([g1c, g1c], axis=0)
    wb[:, G2_O:ONES_O] = i["g2_w"].astype(f).T  # [128, 256]
    wb[:, ONES_O:OB4_O] = 1.0
    ob = i["out_b"].astype(f)
    for Mb in range(2):
        for s in range(2):
            wb[:, OB4_O + Mb * 2 + s] = ob[Mb * 128:(Mb + 1) * 128]

    vb = np.zeros((128, VB), dtype=f)
    vb[:, 0] = np.tile(bias1, 2)
    vb[:, 1] = np.tile(bias2, 2)
    vb[:, 2] = hb
    vb[:, 3] = i["g2_b"].astype(f)[0:128]
    vb[:, 4] = i["g2_b"].astype(f)[128:256]
    return wb.astype(NPBF16), vb


def build_kernel(ctx: ExitStack, tc: tile.TileContext, out_d, x_d, wb_d, vb_d):
    nc = tc.nc
    DW = MARG + P + MARG  # 4228

    consts = ctx.enter_context(tc.tile_pool(name="consts", bufs=1))
    xpool = ctx.enter_context(tc.tile_pool(name="xp", bufs=4))
    bigs = ctx.enter_context(tc.tile_pool(name="bigs", bufs=1))
    small = ctx.enter_context(tc.tile_pool(name="small", bufs=1))
    outp = ctx.enter_context(tc.tile_pool(name="outp", bufs=4))
    ps_main = ctx.enter_context(tc.tile_pool(name="psm", bufs=3, space="PSUM"))
    ps_aux = ctx.enter_context(tc.tile_pool(name="psa", bufs=1, space="PSUM"))
    ps_mlp = ctx.enter_context(tc.tile_pool(name="psl", bufs=1, space="PSUM"))
    dpool = ctx.enter_context(tc.tile_pool(name="dram", bufs=1, space="DRAM"))

    wsb = consts.tile([128, WB], BF16, tag="wsb")
    nc.sync.dma_start(wsb[:], wb_d[:])
    vsb = consts.tile([128, VB], F32, tag="vsb")
    nc.sync.dma_start(vsb[:], vb_d[:])

    def w1ap(t9, kb):
        o = W1_O + (t9 * 2 + kb) * 64
        return wsb[:, o:o + 64]

    def w1cap(e, dy, kb):
        o = W1C_O + ((e * 3 + dy) * 2 + kb) * 64
        return wsb[:, o:o + 64]

    def w2ap(s, t9):
        o = W2_O + t9 * 64
        return wsb[64 * s:64 * s + 64, o:o + 64]

    def w2cap(s, e, dy):
        o = W2C_O + (e * 3 + dy) * 64
        return wsb[64 * s:64 * s + 64, o:o + 64]

    # x tiles: [s][kb] -> [128, DW] bf16, data at [MARG, MARG+P)
    xs = [[None, None], [None, None]]
    for s in range(S):
        for kb in range(2):
            t = xpool.tile([128, DW], BF16, tag="xt")
            nc.vector.memset(t[:, 0:MARG], 0.0)
            nc.vector.memset(t[:, MARG + P:DW], 0.0)
            nc.sync.dma_start(t[:, MARG:MARG + P], x_d[s, kb])
            xs[s][kb] = t

    y1 = bigs.tile([128, DW], BF16, tag="y1")
    nc.vector.memset(y1[:, 0:MARG], 0.0)
    nc.vector.memset(y1[:, MARG + P:DW], 0.0)
    ef = bigs.tile([128, P], BF16, tag="ef")
    scratch = bigs.tile([128, P], BF16, tag="scr")

    xp = small.tile([128, 4], F32, tag="xp")       # raw x sums, col = s*2+kb
    ep = small.tile([128, 4], F32, tag="ep")       # ef tile sums
    eps_sum = small.tile([128, 1], F32, tag="epsum")
    xpb = small.tile([128, 4], BF16, tag="xpb")
    epb = small.tile([128, 1], BF16, tag="epb")
    hb16 = small.tile([128, 2], BF16, tag="hb16")
    gate = small.tile([128, 4], BF16, tag="gate")   # col = Mb*2+s
    gob = small.tile([128, 4], F32, tag="gob")
    gT = small.tile([1, 512], BF16, tag="gT")       # transposed gate rows
    c1sb = small.tile([128, 128], F32, tag="c1sb")  # conv1 edge corrections
    c2sb = small.tile([128, 128], F32, tag="c2sb")
    bc = small.tile([128, 256], BF16, tag="bc")     # gate broadcast
    sw = small.tile([128, 256], BF16, tag="sw")     # gate-scaled out_w

    # ---- conv1 edge-correction matmuls: corr[o, r] for left(c=0)/right(c=63)
    c1ps = ps_aux.tile([128, 128], F32, tag="corr")
    for s in range(S):
        for e in range(2):
            first = True
            for dy in range(3):
                for kb in range(2):
                    if e == 0:
                        off = MARG + (dy - 2) * 64 + 63
                    else:
                        off = MARG + dy * 64
                    # strided view: count 64, step 64 starting at off
                    rhs = xs[s][kb][:, off:off + 63 * 64 + 1:64]
                    nc.tensor.matmul(
                        c1ps[64 * s:64 * s + 64, 64 * e:64 * e + 64],
                        w1cap(e, dy, kb), rhs,
                        start=first, stop=(dy == 2 and kb == 1))
                    first = False
    nc.scalar.copy(c1sb[:], c1ps[:])

    # ---- conv1 main: y1 = relu(conv(x) + bias1), samples on col groups
    n_tiles = P // NT
    for it in range(n_tiles):
        t0 = it * NT
        ps = ps_main.tile([128, NT], F32, tag="mm")
        for h in range(2):
            n0 = t0 + h * 512
            for s in range(S):
                first = True
                for kb in range(2):
                    for t9 in range(9):
                        dy, dx = t9 // 3 - 1, t9 % 3 - 1
                        off = MARG + n0 + dy * 64 + dx
                        nc.tensor.matmul(
                            ps[64 * s:64 * s + 64, h * 512:h * 512 + 512],
                            w1ap(t9, kb),
                            xs[s][kb][:, off:off + 512],
                            start=first, stop=(kb == 1 and t9 == 8))
                        first = False
        # apply edge corrections: cols c=0 and c=63 of each row in this tile
        r0 = t0 // 64
        nrow = NT // 64
        psr = ps[:].rearrange("p (r c) -> p r c", c=64)
        nc.vector.tensor_add(psr[:, :, 0], psr[:, :, 0], c1sb[:, r0:r0 + nrow])
        nc.vector.tensor_add(psr[:, :, 63], psr[:, :, 63],
                             c1sb[:, 64 + r0:64 + r0 + nrow])
        nc.scalar.activation(y1[:, MARG + t0:MARG + t0 + NT], ps[:], AF.Relu,
                             bias=vsb[:, 0:1])

    # ---- x pooling (raw sums) via tensor_scalar accumulate
    for s in range(S):
        for kb in range(2):
            nc.vector.tensor_scalar(
                scratch[:], xs[s][kb][:, MARG:MARG + P], 1.0, 0.0,
                op0=ALU.mult, op1=ALU.add,
                accum_out=xp[:, s * 2 + kb:s * 2 + kb + 1])

    # ---- conv2 edge corrections on y1
    c2ps = ps_aux.tile([128, 128], F32, tag="corr")
    for s in range(S):
        for e in range(2):
            first = True
            for dy in range(3):
                if e == 0:
                    off = MARG + (dy - 2) * 64 + 63
                else:
                    off = MARG + dy * 64
                rhs = y1[64 * s:64 * s + 64, off:off + 63 * 64 + 1:64]
                nc.tensor.matmul(
                    c2ps[64 * s:64 * s + 64, 64 * e:64 * e + 64],
                    w2cap(s, e, dy), rhs,
                    start=first, stop=(dy == 2))
                first = False
    nc.scalar.copy(c2sb[:], c2ps[:])

    # ---- conv2 main: ef = relu(conv(y1) + bias2), diagonal tile_position
    for it in range(n_tiles):
        t0 = it * NT
        ps = ps_main.tile([128, NT], F32, tag="mm")
        for h in range(2):
            n0 = t0 + h * 512
            for s in range(S):
                first = True
                for t9 in range(9):
                    dy, dx = t9 // 3 - 1, t9 % 3 - 1
                    off = MARG + n0 + dy * 64 + dx
                    nc.tensor.matmul(
                        ps[64 * s:64 * s + 64, h * 512:h * 512 + 512],
                        w2ap(s, t9),
                        y1[64 * s:64 * s + 64, off:off + 512],
                        start=first, stop=(t9 == 8))
                    first = False
        r0 = t0 // 64
        nrow = NT // 64
        psr = ps[:].rearrange("p (r c) -> p r c", c=64)
        nc.vector.tensor_add(psr[:, :, 0], psr[:, :, 0], c2sb[:, r0:r0 + nrow])
        nc.vector.tensor_add(psr[:, :, 63], psr[:, :, 63],
                             c2sb[:, 64 + r0:64 + r0 + nrow])
        nc.scalar.activation(ef[:, t0:t0 + NT], ps[:], AF.Relu,
                             bias=vsb[:, 1:2], accum_out=ep[:, it:it + 1])

    # ---- gate MLP
    nc.vector.reduce_sum(eps_sum[:, 0:1], ep[:], axis=mybir.AxisListType.X)
    nc.scalar.copy(xpb[:], xp[:])
    nc.scalar.copy(epb[:], eps_sum[:])

    hps = ps_mlp.tile([128, 4], F32, tag="mlp")
    nc.tensor.matmul(hps[:, 0:2], wsb[:, G1A_O:G1A_O + 128], xpb[:, 0:4:2],
                     start=True, stop=False, skip_group_check=True)
    nc.tensor.matmul(hps[:, 0:2], wsb[:, G1B_O:G1B_O + 128], xpb[:, 1:4:2],
                     start=False, stop=False, skip_group_check=True)
    for s in range(S):
        nc.tensor.matmul(hps[:, s:s + 1],
                         wsb[64 * s:64 * s + 64, G1C_O:G1C_O + 128],
                         epb[64 * s:64 * s + 64, 0:1],
                         start=False, stop=(s == 1), skip_group_check=True)
    nc.scalar.activation(hb16[:], hps[:, 0:2], AF.Relu, bias=vsb[:, 2:3])

    gps = ps_mlp.tile([128, 4], F32, tag="mlp")
    for Mb in range(2):
        nc.tensor.matmul(gps[:, Mb * 2:Mb * 2 + 2],
                         wsb[:, G2_O + Mb * 128:G2_O + Mb * 128 + 128],
                         hb16[:], start=True, stop=True)
    for Mb in range(2):
        nc.scalar.activation(gate[:, Mb * 2:Mb * 2 + 2], gps[:, Mb * 2:Mb * 2 + 2],
                             AF.Sigmoid, bias=vsb[:, 3 + Mb:4 + Mb])
    nc.vector.tensor_mul(gob[:], gate[:], wsb[:, OB4_O:OB4_O + 4])

    # gate -> free-dim rows (via DRAM bounce), then broadcast across
    # partitions via ones-matmul
    gdram = dpool.tile([128, 4], BF16, tag="gdram")
    nc.sync.dma_start(gdram[:], gate[:])
    for j in range(4):
        nc.sync.dma_start(gT[0:1, j * 128:(j + 1) * 128], gdram[:, j:j + 1])
    bps = ps_aux.tile([128, 256], F32, tag="corr")
    for s in range(S):
        for Mb in range(2):
            j = Mb * 2 + s
            nc.tensor.matmul(bps[64 * s:64 * s + 64, Mb * 128:Mb * 128 + 128],
                             wsb[0:1, ONES_O:ONES_O + 64],
                             gT[0:1, j * 128:(j + 1) * 128],
                             start=True, stop=True)
    nc.scalar.copy(bc[:], bps[:])
    nc.vector.tensor_mul(sw[:], wsb[:, OW_O:OW_O + 256], bc[:])

    # ---- out conv + gated residual: out = x + (sw.T @ ef + gob)
    for it in range(n_tiles):
        t0 = it * NT
        for Mb in range(2):
            for s in range(S):
                pso = ps_main.tile([128, NT], F32, tag="mm")
                for h in range(2):
                    n0 = t0 + h * 512
                    nc.tensor.matmul(
                        pso[:, h * 512:h * 512 + 512],
                        sw[64 * s:64 * s + 64, Mb * 128:Mb * 128 + 128],
                        ef[64 * s:64 * s + 64, n0:n0 + 512],
                        start=True, stop=True)
                osb = outp.tile([128, NT], BF16, tag="osb")
                j = Mb * 2 + s
                nc.vector.scalar_tensor_tensor(
                    osb[:], pso[:], gob[:, j:j + 1],
                    xs[s][Mb][:, MARG + t0:MARG + t0 + NT],
                    op0=ALU.add, op1=ALU.add)
                nc.sync.dma_start(out_d[s, Mb, :, t0:t0 + NT], osb[:])


_CACHED = None


def _get_nc():
    global _CACHED
    if _CACHED is None:
        nc = bacc.Bacc("TRN2", target_bir_lowering=False, debug=False,
                       num_devices=NCORES)
        x_d = nc.dram_tensor("x", [S, 2, 128, P], BF16, kind="ExternalInput").ap()
        wb_d = nc.dram_tensor("wb", [128, WB], BF16, kind="ExternalInput").ap()
        vb_d = nc.dram_tensor("vb", [128, VB], F32, kind="ExternalInput").ap()
        out_d = nc.dram_tensor("out", [S, 2, 128, P], BF16,
                               kind="ExternalOutput").ap()
        with tile.TileContext(nc) as tc:
            with ExitStack() as ctx:
                build_kernel(ctx, tc, out_d, x_d, wb_d, vb_d)
        nc.compile()
        _CACHED = nc
    return _CACHED


LAST_EXEC_NS = None


def kernel(**inputs):
    global LAST_EXEC_NS
    import os

    nc = _get_nc()
    x = np.asarray(inputs["x"], dtype=np.float32)
    # [16,256,64,64] -> [8 cores, 2 samples, 2 kb, 128, 4096] bf16
    x8 = np.ascontiguousarray(
        x.reshape(NCORES, S, 2, 128, P)).astype(NPBF16)
    wb, vb = _prep_weights(inputs)
    in_maps = [{"x": x8[c], "wb": wb, "vb": vb} for c in range(NCORES)]
    trace = bool(os.environ.get("KERNEL_TRACE"))
    res = bass_utils.run_bass_kernel_spmd(
        nc, in_maps, core_ids=list(range(NCORES)), trace=trace)
    LAST_EXEC_NS = res.exec_time_ns
    out = np.empty((B, C, H, W), dtype=np.float32)
    for c in range(NCORES):
        o = np.asarray(res.results[c]["out"]).astype(np.float32)
        out[2 * c:2 * c + 2] = o.reshape(S, C, H, W)
    return out


# revision 11
# speedup vs baseline: 2.9841x; 2.9841x over previous
import sys

sys.path.insert(0, "/opt/trn_rl_repo")

from contextlib import ExitStack

import ml_dtypes
import numpy as np

import concourse.bass as bass
import concourse.bacc as bacc
import concourse.tile as tile
from concourse import bass_utils, mybir

BF16 = mybir.dt.bfloat16
F32 = mybir.dt.float32
AF = mybir.ActivationFunctionType
ALU = mybir.AluOpType
NPBF16 = ml_dtypes.bfloat16

B, C, H, W = 16, 256, 64, 64
Cq = C // 4        # 64
Gh = C // 2        # 128
P = H * W          # 4096 spatial positions per sample
MARG = 66          # zero margin (elements) on each side of the flat spatial dim
NCORES = 8
S = 2              # samples per core
NT = 1024          # psum tile free size
EPS = 1e-5

# ---- weight blob column offsets (bf16 blob, [128, WB]) ----
W1_O = 0                       # [9 taps][2 kb][64 o]            -> 1152
W1C_O = W1_O + 9 * 2 * 64      # [2 edge][3 dy][2 kb][64 o]      -> 768
W2_O = W1C_O + 2 * 3 * 2 * 64  # [9 taps][64 o] (dup both halves)-> 576
W2C_O = W2_O + 9 * 64          # [2 edge][3 dy][64 o] (dup)      -> 384
OW_O = W2C_O + 2 * 3 * 64      # [256] out_w.T (dup)             -> 256
G1A_O = OW_O + 256             # [128]
G1B_O = G1A_O + 128            # [128]
G1C_O = G1B_O + 128            # [128] (dup halves)
G2_O = G1C_O + 128             # [256]
ONES_O = G2_O + 256            # [64] all ones
OB4_O = ONES_O + 64            # [4]  out_b per (Mb*2+s) col
WB = OB4_O + 4

# f32 vector blob [128, VB]: bias1, bias2, hb, g2b(2 cols)
VB = 5


def _prep_weights(i):
    f = np.float32
    inv1 = (i["bn1_g"] / np.sqrt(i["bn1_v"] + EPS)).astype(f)          # [64]
    W1f = (i["ec1_w"] * inv1[:, None, None, None]).astype(f)           # [64,256,3,3]
    bias1 = (inv1 * (i["ec1_b"] - i["bn1_m"]) + i["bn1_b"]).astype(f)  # [64]

    inv2 = (i["bn2_g"] / np.sqrt(i["bn2_v"] + EPS)).astype(f)
    W2f = (i["ec2_w"] * inv2[:, None, None, None]).astype(f)           # [64,64,3,3]
    bias2 = (inv2 * (i["ec2_b"] - i["bn2_m"]) + i["bn2_b"]).astype(f)

    pre = (i["gbn_g"] / np.sqrt(i["gbn_v"] + EPS)).astype(f)           # [128]
    G1f = (i["g1_w"] * pre[:, None] / P).astype(f)                     # [128,320]
    hb = (pre * (i["g1_b"] - i["gbn_m"]) + i["gbn_b"]).astype(f)       # [128]

    wb = np.zeros((128, WB), dtype=f)
    # w1: [k, t9, kb, o] = W1f[o, kb*128+k, dy, dx]
    a = np.transpose(W1f.reshape(64, 2, 128, 3, 3), (2, 3, 4, 1, 0))  # [k,dy,dx,kb,o]
    wb[:, W1_O:W1C_O] = a.reshape(128, 9 * 2 * 64)
    # w1c: [k, e, dy, kb, o] = -W1f[o, kb*128+k, dy, dx_e], dx_e = (0, 2)
    an = -np.transpose(W1f.reshape(64, 2, 128, 3, 3), (2, 3, 1, 0, 4))  # [k,dy,kb,o,dx]
    w1c = np.stack([an[..., 0], an[..., 2]], axis=1)  # [k, e, dy, kb, o]
    wb[:, W1C_O:W2_O] = w1c.reshape(128, 768)
    # w2: [64s+k, t9, o] = W2f[o, k, dy, dx]
    b2 = np.transpose(W2f, (1, 2, 3, 0)).reshape(64, 9, 64)  # [k, t9, o]
    wb[:, W2_O:W2C_O] = np.concatenate([b2, b2], axis=0).reshape(128, 576)
    # w2c: [64s+k, e, dy, o] = -W2f[o, k, dy, dx_e]
    b2n = -np.transpose(W2f, (1, 2, 3, 0))  # [k, dy, dx, o]
    w2c = np.stack([b2n[:, :, 0, :], b2n[:, :, 2, :]], axis=1)  # [k, e, dy, o]
    wb[:, W2C_O:OW_O] = np.concatenate([w2c, w2c], axis=0).reshape(128, 384)
    # out_w.T dup
    owT = i["out_w"].astype(f).T  # [64, 256]
    wb[:, OW_O:G1A_O] = np.concatenate([owT, owT], axis=0)
    # gating
    wb[:, G1A_O:G1B_O] = G1f[:, 0:128].T
    wb[:, G1B_O:G1C_O] = G1f[:, 128:256].T
    g1c = G1f[:, 256:320].T  # [64, 128]
    wb[:, G1C_O:G2_O] = np.concatenate([g1c, g1c], axis=0)
    wb[:, G2_O:ONES_O] = i["g2_w"].astype(f).T  # [128, 256]
    wb[:, ONES_O:OB4_O] = 1.0
    ob = i["out_b"].astype(f)
    for Mb in range(2):
        for s in range(2):
            wb[:, OB4_O + Mb * 2 + s] = ob[Mb * 128:(Mb + 1) * 128]

    vb = np.zeros((128, VB), dtype=f)
    vb[:, 0] = np.tile(bias1, 2)
    vb[:, 1] = np.tile(bias2, 2)
    vb[:, 2] = hb
    vb[:, 3] = i["g2_b"].astype(f)[0:128]
    vb[:, 4] = i["g2_b"].astype(f)[128:256]
    return wb.astype(NPBF16), vb


def build_kernel(ctx: ExitStack, tc: tile.TileContext, out_d, x_d, wb_d, vb_d):
    nc = tc.nc
    DW = MARG + P + MARG  # 4228

    consts = ctx.enter_context(tc.tile_pool(name="consts", bufs=1))
    xpool = ctx.enter_context(tc.tile_pool(name="xp", bufs=4))
    bigs = ctx.enter_context(tc.tile_pool(name="bigs", bufs=1))
    small = ctx.enter_context(tc.tile_pool(name="small", bufs=1))
    outp = ctx.enter_context(tc.tile_pool(name="outp", bufs=4))
    ps_main = ctx.enter_context(tc.tile_pool(name="psm", bufs=3, space="PSUM"))
    ps_aux = ctx.enter_context(tc.tile_pool(name="psa", bufs=1, space="PSUM"))
    ps_mlp = ctx.enter_context(tc.tile_pool(name="psl", bufs=1, space="PSUM"))
    dpool = ctx.enter_context(tc.tile_pool(name="dram", bufs=1, space="DRAM"))

    wsb = consts.tile([128, WB], BF16, tag="wsb")
    nc.sync.dma_start(wsb[:], wb_d[:])
    vsb = consts.tile([128, VB], F32, tag="vsb")
    nc.sync.dma_start(vsb[:], vb_d[:])

    def w1ap(t9, kb):
        o = W1_O + (t9 * 2 + kb) * 64
        return wsb[:, o:o + 64]

    def w1cap(e, dy, kb):
        o = W1C_O + ((e * 3 + dy) * 2 + kb) * 64
        return wsb[:, o:o + 64]

    def w2ap(s, t9):
        o = W2_O + t9 * 64
        return wsb[64 * s:64 * s + 64, o:o + 64]

    def w2cap(s, e, dy):
        o = W2C_O + (e * 3 + dy) * 64
        return wsb[64 * s:64 * s + 64, o:o + 64]

    # x tiles: [s][kb] -> [128, DW] bf16, data at [MARG, MARG+P)
    xs = [[None, None], [None, None]]
    for s in range(S):
        for kb in range(2):
            t = xpool.tile([128, DW], BF16, tag="xt")
            nc.vector.memset(t[:, 0:MARG], 0.0)
            nc.vector.memset(t[:, MARG + P:DW], 0.0)
            nc.sync.dma_start(t[:, MARG:MARG + P], x_d[s, kb])
            xs[s][kb] = t

    y1 = bigs.tile([128, DW], BF16, tag="y1")
    nc.vector.memset(y1[:, 0:MARG], 0.0)
    nc.vector.memset(y1[:, MARG + P:DW], 0.0)
    ef = bigs.tile([128, P], BF16, tag="ef")
    scratch = bigs.tile([128, P], BF16, tag="scr")

    xp = small.tile([128, 4], F32, tag="xp")       # raw x sums, col = s*2+kb
    ep = small.tile([128, 4], F32, tag="ep")       # ef tile sums
    eps_sum = small.tile([128, 1], F32, tag="epsum")
    xpb = small.tile([128, 4], BF16, tag="xpb")
    epb = small.tile([128, 1], BF16, tag="epb")
    hb16 = small.tile([128, 2], BF16, tag="hb16")
    gate = small.tile([128, 4], BF16, tag="gate")   # col = Mb*2+s
    gob = small.tile([128, 4], F32, tag="gob")
    gT = small.tile([1, 512], BF16, tag="gT")       # transposed gate rows
    c1sb = small.tile([128, 128], F32, tag="c1sb")  # conv1 edge corrections
    c2sb = small.tile([128, 128], F32, tag="c2sb")
    bc = small.tile([128, 256], BF16, tag="bc")     # gate broadcast
    sw = small.tile([128, 256], BF16, tag="sw")     # gate-scaled out_w

    # ---- conv1 edge-correction matmuls: corr[o, r] for left(c=0)/right(c=63)
    c1ps = ps_aux.tile([128, 128], F32, tag="corr")
    for s in range(S):
        for e in range(2):
            first = True
            for dy in range(3):
                for kb in range(2):
                    if e == 0:
                        off = MARG + (dy - 2) * 64 + 63
                    else:
                        off = MARG + dy * 64
                    # strided view: count 64, step 64 starting at off
                    rhs = xs[s][kb][:, off:off + 63 * 64 + 1:64]
                    nc.tensor.matmul(
                        c1ps[64 * s:64 * s + 64, 64 * e:64 * e + 64],
                        w1cap(e, dy, kb), rhs,
                        start=first, stop=(dy == 2 and kb == 1))
                    first = False
    nc.scalar.copy(c1sb[:], c1ps[:])

    # ---- conv1 main: y1 = relu(conv(x) + bias1), samples on col groups
    n_tiles = P // NT
    for it in range(n_tiles):
        t0 = it * NT
        ps = ps_main.tile([128, NT], F32, tag="mm")
        for h in range(2):
            n0 = t0 + h * 512
            for s in range(S):
                first = True
                for kb in range(2):
                    for t9 in range(9):
                        dy, dx = t9 // 3 - 1, t9 % 3 - 1
                        off = MARG + n0 + dy * 64 + dx
                        nc.tensor.matmul(
                            ps[64 * s:64 * s + 64, h * 512:h * 512 + 512],
                            w1ap(t9, kb),
                            xs[s][kb][:, off:off + 512],
                            start=first, stop=(kb == 1 and t9 == 8))
                        first = False
        # apply edge corrections: cols c=0 and c=63 of each row in this tile
        r0 = t0 // 64
        nrow = NT // 64
        psr = ps[:].rearrange("p (r c) -> p r c", c=64)
        nc.vector.tensor_add(psr[:, :, 0], psr[:, :, 0], c1sb[:, r0:r0 + nrow])
        nc.vector.tensor_add(psr[:, :, 63], psr[:, :, 63],
                             c1sb[:, 64 + r0:64 + r0 + nrow])
        nc.scalar.activation(y1[:, MARG + t0:MARG + t0 + NT], ps[:], AF.Relu,
                             bias=vsb[:, 0:1])

    # ---- x pooling (raw sums) via tensor_scalar accumulate
    for s in range(S):
        for kb in range(2):
            nc.vector.tensor_scalar(
                scratch[:], xs[s][kb][:, MARG:MARG + P], 1.0, 0.0,
                op0=ALU.mult, op1=ALU.add,
                accum_out=xp[:, s * 2 + kb:s * 2 + kb + 1])

    # ---- conv2 edge corrections on y1
    c2ps = ps_aux.tile([128, 128], F32, tag="corr")
    for s in range(S):
        for e in range(2):
            first = True
            for dy in range(3):
                if e == 0:
                    off = MARG + (dy - 2) * 64 + 63
                else:
                    off = MARG + dy * 64
                rhs = y1[64 * s:64 * s + 64, off:off + 63 * 64 + 1:64]
                nc.tensor.matmul(
                    c2ps[64 * s:64 * s + 64, 64 * e:64 * e + 64],
                    w2cap(s, e, dy), rhs,
                    start=first, stop=(dy == 2))
                first = False
    nc.scalar.copy(c2sb[:], c2ps[:])

    # ---- conv2 main: ef = relu(conv(y1) + bias2), diagonal tile_position
    for it in range(n_tiles):
        t0 = it * NT
        ps = ps_main.tile([128, NT], F32, tag="mm")
        for h in range(2):
            n0 = t0 + h * 512
            for s in range(S):
                first = True
                for t9 in range(9):
                    dy, dx = t9 // 3 - 1, t9 % 3 - 1
                    off = MARG + n0 + dy * 64 + dx
                    nc.tensor.matmul(
                        ps[64 * s:64 * s + 64, h * 512:h * 512 + 512],
                        w2ap(s, t9),
                        y1[64 * s:64 * s + 64, off:off + 512],
                        start=first, stop=(t9 == 8))
                    first = False
        r0 = t0 // 64
        nrow = NT // 64
        psr = ps[:].rearrange("p (r c) -> p r c", c=64)
        nc.vector.tensor_add(psr[:, :, 0], psr[:, :, 0], c2sb[:, r0:r0 + nrow])
        nc.vector.tensor_add(psr[:, :, 63], psr[:, :, 63],
                             c2sb[:, 64 + r0:64 + r0 + nrow])
        nc.scalar.activation(ef[:, t0:t0 + NT], ps[:], AF.Relu,
                             bias=vsb[:, 1:2], accum_out=ep[:, it:it + 1])

    # ---- gate MLP
    nc.vector.reduce_sum(eps_sum[:, 0:1], ep[:], axis=mybir.AxisListType.X)
    nc.scalar.copy(xpb[:], xp[:])
    nc.scalar.copy(epb[:], eps_sum[:])

    hps = ps_mlp.tile([128, 4], F32, tag="mlp")
    nc.tensor.matmul(hps[:, 0:2], wsb[:, G1A_O:G1A_O + 128], xpb[:, 0:4:2],
                     start=True, stop=False, skip_group_check=True)
    nc.tensor.matmul(hps[:, 0:2], wsb[:, G1B_O:G1B_O + 128], xpb[:, 1:4:2],
                     start=False, stop=False, skip_group_check=True)
    for s in range(S):
        nc.tensor.matmul(hps[:, s:s + 1],
                         wsb[64 * s:64 * s + 64, G1C_O:G1C_O + 128],
                         epb[64 * s:64 * s + 64, 0:1],
                         start=False, stop=(s == 1), skip_group_check=True)
    nc.scalar.activation(hb16[:], hps[:, 0:2], AF.Relu, bias=vsb[:, 2:3])

    gps = ps_mlp.tile([128, 4], F32, tag="mlp")
    for Mb in range(2):
        nc.tensor.matmul(gps[:, Mb * 2:Mb * 2 + 2],
                         wsb[:, G2_O + Mb * 128:G2_O + Mb * 128 + 128],
                         hb16[:], start=True, stop=True)
    for Mb in range(2):
        nc.scalar.activation(gate[:, Mb * 2:Mb * 2 + 2], gps[:, Mb * 2:Mb * 2 + 2],
                             AF.Sigmoid, bias=vsb[:, 3 + Mb:4 + Mb])
    nc.vector.tensor_mul(gob[:], gate[:], wsb[:, OB4_O:OB4_O + 4])

    # gate -> free-dim rows (via DRAM bounce), then broadcast across
    # partitions via ones-matmul
    gdram = dpool.tile([128, 4], BF16, tag="gdram")
    nc.sync.dma_start(gdram[:], gate[:])
    for j in range(4):
        nc.sync.dma_start(gT[0:1, j * 128:(j + 1) * 128], gdram[:, j:j + 1])
    bps = ps_aux.tile([128, 256], F32, tag="corr")
    for s in range(S):
        for Mb in range(2):
            j = Mb * 2 + s
            nc.tensor.matmul(bps[64 * s:64 * s + 64, Mb * 128:Mb * 128 + 128],
                             wsb[0:1, ONES_O:ONES_O + 64],
                             gT[0:1, j * 128:(j + 1) * 128],
                             start=True, stop=True)
    nc.scalar.copy(bc[:], bps[:])
    nc.vector.tensor_mul(sw[:], wsb[:, OW_O:OW_O + 256], bc[:])

    # ---- out conv + gated residual: out = x + (sw.T @ ef + gob)
    for it in range(n_tiles):
        t0 = it * NT
        for Mb in range(2):
            for s in range(S):
                pso = ps_main.tile([128, NT], F32, tag="mm")
                for h in range(2):
                    n0 = t0 + h * 512
                    nc.tensor.matmul(
                        pso[:, h * 512:h * 512 + 512],
                        sw[64 * s:64 * s + 64, Mb * 128:Mb * 128 + 128],
                        ef[64 * s:64 * s + 64, n0:n0 + 512],
                        start=True, stop=True)
                osb = outp.tile([128, NT], BF16, tag="osb")
                j = Mb * 2 + s
                nc.vector.scalar_tensor_tensor(
                    osb[:], pso[:], gob[:, j:j + 1],
                    xs[s][Mb][:, MARG + t0:MARG + t0 + NT],
                    op0=ALU.add, op1=ALU.add)
                nc.sync.dma_start(out_d[s, Mb, :, t0:t0 + NT], osb[:])


_CACHED = None
_RUNNER = None


def _get_runner():
    """Build (once) a cached jitted 8-core dispatcher for the Bass NEFF."""
    global _RUNNER
    if _RUNNER is not None:
        return _RUNNER
    import jax
    import jax.numpy as jnp
    from jax.sharding import Mesh, NamedSharding, PartitionSpec
    from jax.experimental.shard_map import shard_map
    from concourse import bass2jax

    nc = _get_nc()
    bass2jax.install_neuronx_cc_hook()
    pname = nc.partition_id_tensor.name if nc.partition_id_tensor else None
    in_names, out_names, out_avals = [], [], []
    for alloc in nc.m.functions[0].allocations:
        if not isinstance(alloc, mybir.MemoryLocationSet):
            continue
        name = alloc.memorylocations[0].name
        if alloc.kind == "ExternalInput":
            if name != pname:
                in_names.append(name)
        elif alloc.kind == "ExternalOutput":
            out_names.append(name)
            out_avals.append(jax.core.ShapedArray(
                tuple(alloc.tensor_shape), mybir.dt.np(alloc.dtype)))
    n_params = len(in_names)
    all_names = in_names + out_names + ([pname] if pname else [])

    def _body(*args):
        operands = list(args)
        if pname:
            operands.append(bass2jax.partition_id_tensor())
        return tuple(bass2jax._bass_exec_p.bind(
            *operands, out_avals=tuple(out_avals), in_names=tuple(all_names),
            out_names=tuple(out_names), lowering_input_output_aliases=(),
            sim_require_finite=True, sim_require_nnan=True, nc=nc))

    mesh = Mesh(np.asarray(jax.devices()[:NCORES]), ("core",))
    sh = NamedSharding(mesh, PartitionSpec("core"))
    n_outs = len(out_names)
    sharded = jax.jit(
        shard_map(_body, mesh=mesh,
                  in_specs=(PartitionSpec("core"),) * (n_params + n_outs),
                  out_specs=(PartitionSpec("core"),) * n_outs,
                  check_rep=False),
        donate_argnums=tuple(range(n_params, n_params + n_outs)),
        keep_unused=True)
    zmakers = [
        jax.jit(lambda a=a: jnp.zeros((NCORES * a.shape[0],) + a.shape[1:],
                                      a.dtype), out_shardings=sh)
        for a in out_avals]

    def run(x8, wb, vb):
        # x8: [16, 2, 128, P] bf16 (core-major); wb/vb replicated per core
        feed = {
            "x": jax.device_put(x8, sh),
            "wb": jax.device_put(
                np.broadcast_to(wb[None], (NCORES,) + wb.shape).reshape(
                    NCORES * 128, WB), sh),
            "vb": jax.device_put(
                np.broadcast_to(vb[None], (NCORES,) + vb.shape).reshape(
                    NCORES * 128, VB), sh),
        }
        zs = [zm() for zm in zmakers]
        out, = sharded(*[feed[n] for n in in_names], *zs)
        return np.asarray(jax.device_get(out))

    _RUNNER = run
    return run


def _get_nc():
    global _CACHED
    if _CACHED is None:
        nc = bacc.Bacc("TRN2", target_bir_lowering=False, debug=False,
                       num_devices=NCORES)
        x_d = nc.dram_tensor("x", [S, 2, 128, P], BF16, kind="ExternalInput").ap()
        wb_d = nc.dram_tensor("wb", [128, WB], BF16, kind="ExternalInput").ap()
        vb_d = nc.dram_tensor("vb", [128, VB], F32, kind="ExternalInput").ap()
        out_d = nc.dram_tensor("out", [S, 2, 128, P], BF16,
                               kind="ExternalOutput").ap()
        with tile.TileContext(nc) as tc:
            with ExitStack() as ctx:
                build_kernel(ctx, tc, out_d, x_d, wb_d, vb_d)
        nc.compile()
        _CACHED = nc
    return _CACHED


LAST_EXEC_NS = None


def kernel(**inputs):
    run = _get_runner()
    x = np.asarray(inputs["x"], dtype=np.float32)
    # [16,256,64,64] -> [16 core-major shards of [2 kb, 128, 4096]] bf16
    x8 = np.ascontiguousarray(x.reshape(NCORES * S, 2, 128, P)).astype(NPBF16)
    wb, vb = _prep_weights(inputs)
    o = run(x8, wb, vb)  # [16, 2, 128, P] bf16
    return o.astype(np.float32).reshape(B, C, H, W)
